# revision 1
# baseline (speedup 1.0000x reference)
"""Trainium2 Bass kernel for nn_BinarizedCifar10MLP — v2.

Data-parallel over batch (8192/8 = 1024 rows/core), feature-major layout.

vs v1 (849us):
  - L1 lo-pass in fp8 DoubleRow: x = fp16(x) + 2^-12 * e4m3(2^12*(x-fp16(x))).
    Hi pass: bf16 +-1 weights x fp16 rhs (1 cyc/row). Lo pass: e5m2 +-2^-12
    weights x e4m3 rhs, DoubleRow (0.5 cyc/row), same PSUM group. 1.5 cyc/row
    total vs 2.0.
  - L2/L3 fully fp8 DoubleRow (weights pre-signed to DRAM during L1).
  - Chunked x DMA (PE starts ~6us in, not ~45us).
  - BatchNorm stat AllReduces split in two chunks per layer (m-tiles 0..13 /
    14..15) and pipelined against the tail of each layer's matmuls; next
    layer consumes k-tiles in chunk order so the last AR hides behind the
    PSUM-bank runway.
  - Weight sign conversion spread across ACT; sign waves on DVE+ACT.
"""

import sys

sys.path.insert(0, "/opt/trn_rl_repo")

import numpy as np
import ml_dtypes

B, D, H, C = 8192, 3 * 32 * 32, 2048, 10
EPS = 1e-5
NCORES = 8
BS = B // NCORES          # 1024 batch rows per core
KD = D // 128             # 24 k-tiles over input dim
KH = H // 128             # 16 k-tiles over hidden dim
NB = BS // 512            # 2 free-dim chunks of 512
MAL = {1: 14, 2: 14, 3: 12}   # per-layer chunk A = m-tiles [0, MAL[l])
XCH = 4                   # x DMA chunks (6 k-tiles each)

_CACHE = {}


def _pcol(l, m, n):
    """parts sum-column for (m, n); sq column = 32 + _pcol."""
    ma = MAL[l]
    return 2 * m + n if m < ma else 2 * ma + 2 * (m - ma) + n


def _build(stage=7, fast=(False, False)):
    import concourse.bacc as bacc
    import concourse.mybir as mybir
    import concourse.tile as tile

    F32 = mybir.dt.float32
    F16 = mybir.dt.float16
    BF16 = mybir.dt.bfloat16
    F8E4 = mybir.dt.float8e4
    F8E5 = mybir.dt.float8e5
    DRM = mybir.MatmulPerfMode.DoubleRow
    ACT = mybir.ActivationFunctionType
    ALU = mybir.AluOpType
    RG = [list(range(NCORES))]

    nc = bacc.Bacc("TRN2", target_bir_lowering=False, debug=False, num_devices=NCORES)

    # ---- I/O ----
    xhi_d = nc.dram_tensor("xT_hi", [D, BS], F16, kind="ExternalInput").ap()
    xlo_d = nc.dram_tensor("xT_lo8", [D, BS], F8E4, kind="ExternalInput").ap()
    w1t_d = nc.dram_tensor("W1T", [D, H], BF16, kind="ExternalInput").ap()
    w2t_d = nc.dram_tensor("W2T", [H, H], BF16, kind="ExternalInput").ap()
    w3t_d = nc.dram_tensor("W3T", [H, H], BF16, kind="ExternalInput").ap()
    CNAMES = ("b1", "g1", "bt1", "b2", "g2", "bt2", "b3", "g3", "bt3")
    cpk_d = nc.dram_tensor("cpk", [128, KH * len(CNAMES)], F32, kind="ExternalInput").ap()
    w4pk_d = nc.dram_tensor("w4pk", [128, C * KH], F16, kind="ExternalInput").ap()
    b4_d = nc.dram_tensor("c_b4", [16, 1], F32, kind="ExternalInput").ap()
    out_d = nc.dram_tensor("outT", [C, BS], F32, kind="ExternalOutput").ap()

    wl_d = {2: w2t_d, 3: w3t_d}

    with tile.TileContext(nc) as tc:
        with (
            tc.tile_pool(name="pconst", bufs=1) as pconst,
            tc.tile_pool(name="pstat", bufs=1) as pstat,
            tc.tile_pool(name="plog", bufs=1) as plog,
            tc.tile_pool(name="pscr", bufs=3) as pscr,
            tc.tile_pool(name="pw", bufs=2) as pw,
            tc.tile_pool(name="pw8", bufs=2) as pw8,
            tc.tile_pool(name="ppre", bufs=3) as ppre,
            tc.tile_pool(name="ps8", bufs=3) as ps8,
            tc.tile_pool(name="pwdr", bufs=4) as pwdr,
            tc.tile_pool(name="ph", bufs=1) as ph,
            tc.tile_pool(name="pa", bufs=1) as pa,
            tc.tile_pool(name="pb", bufs=1) as pb,
            tc.tile_pool(name="ppsum", bufs=8, space="PSUM") as ppsum,
            tc.tile_pool(name="pdram", bufs=16, space="DRAM") as pdram,
        ):
            # ---- warmup AllReduce: absorbs ncfw first-collective staging ----
            wuin = pdram.tile([128, 4], F32, tag="wuin")
            wuout = pdram.tile([128, 4], F32, tag="wuout")
            wusrc = pstat.tile([128, 4], F32, tag="wusrc")
            nc.vector.memset(wusrc[:], 0.0)
            nc.sync.dma_start(wuin[:], wusrc[:])
            nc.gpsimd.collective_compute(
                "AllReduce", ALU.add, replica_groups=RG,
                ins=[wuin.opt()], outs=[wuout.opt()])

            # ---- constants ----
            cpk = pconst.tile([128, KH * len(CNAMES)], F32, tag="cpk")
            nc.sync.dma_start(cpk[:], cpk_d)
            cons = {name: cpk[:, i * KH:(i + 1) * KH] for i, name in enumerate(CNAMES)}
            b4s = pconst.tile([16, 1], F32, tag="b4")
            nc.sync.dma_start(b4s[:], b4_d)
            ones10 = pconst.tile([16, 1], F32, tag="ones10")
            nc.vector.memset(ones10[:], 1.0)
            onesC = pconst.tile([1, 16], F32, tag="onesC")
            nc.vector.memset(onesC[:], 1.0)
            w4f = pconst.tile([128, C * KH], F16, tag="w4f")
            nc.sync.dma_start(w4f[:], w4pk_d)

            # ---- x: chunked DMAs so the PE can start early ----
            xhi = pa.tile([128, KD * BS], F16, tag="pa", name="xhi")
            xlo8 = pb.tile([128, KD * BS], F8E4, tag="pb", name="xlo8")
            kc = KD // XCH
            for cch in range(XCH):
                sl = slice(cch * kc * BS, (cch + 1) * kc * BS)
                nc.sync.dma_start(
                    xhi[:, sl].rearrange("p (k c) -> p k c", c=BS),
                    xhi_d[cch * kc * 128:(cch + 1) * kc * 128, :]
                    .rearrange("(k p) c -> p k c", p=128),
                )
                nc.sync.dma_start(
                    xlo8[:, sl].rearrange("p (k c) -> p k c", c=BS),
                    xlo_d[cch * kc * 128:(cch + 1) * kc * 128, :]
                    .rearrange("(k p) c -> p k c", p=128),
                )
            xlo8v = xlo8[:].rearrange("p (k c) -> p k c", c=BS)

            # DRAM scratch for pre-signed fp8 DR weights of L2/L3
            ws8 = {
                l: pdram.tile([128, KH * KH * 128], F8E4, tag=f"ws8_{l}",
                              name=f"ws8_{l}")
                for l in (2, 3)
            }
            # prepass work list: (layer, m, kg) with kg in 0..1 (1024 rows each)
            pre_units = [(l, m, kg) for l in (2, 3) for m in range(KH) for kg in range(2)]

            def emit_prepass(units):
                for (l, m, kg) in units:
                    # all prepass DMAs ride the scalar queue: the out-DMA is
                    # emitted right after its Sign on the same engine, and the
                    # sync queue stays clear for latency-critical AR transfers
                    wst = ppre.tile([128, 1024], BF16, tag="pre", name=f"pre_{l}_{m}_{kg}")
                    src = wl_d[l][kg * 1024:(kg + 1) * 1024, m * 128:(m + 1) * 128]
                    nc.sync.dma_start(
                        wst[:].rearrange("p (j c) -> p j c", j=8),
                        src.rearrange("(j p) c -> p j c", p=128),
                    )
                    s8 = ps8.tile([128, 1024], F8E4, tag="s8", name=f"s8_{l}_{m}_{kg}")
                    nc.scalar.activation(s8[:], wst[:], ACT.Sign)
                    nc.scalar.dma_start(
                        ws8[l][:, m * 2048 + kg * 1024: m * 2048 + (kg + 1) * 1024],
                        s8[:],
                    )

            parts = {}
            gchunk = {}     # (l, 'A'|'B') -> allreduced stats tile
            stats = {}      # (l, 'A'|'B') -> dict of per-chunk stat tiles

            arouts = {}

            def emit_ar_fire(l, chunk, do_sq):
                """DMA parts chunk -> DRAM + trigger AllReduce."""
                nca = 2 * MAL[l]
                c0, c1 = (0, nca) if chunk == "A" else (nca, 32)
                ncol = c1 - c0
                w = 2 * ncol if do_sq else ncol
                arin = pdram.tile([128, w], F32, tag=f"arin{l}{chunk}")
                arout = pdram.tile([128, w], F32, tag=f"arout{l}{chunk}")
                nc.sync.dma_start(arin[:, 0:ncol], parts[l][:, c0:c1])
                if do_sq:
                    nc.sync.dma_start(arin[:, ncol:w], parts[l][:, 32 + c0:32 + c1])
                nc.gpsimd.collective_compute(
                    "AllReduce", ALU.add, replica_groups=RG,
                    ins=[arin.opt()], outs=[arout.opt()])
                arouts[(l, chunk)] = (arout, w)

            def emit_ar_land(l, chunk):
                """Load the allreduced chunk into SBUF (blocks sync until the
                collective completes, so emit only after critical DMAs)."""
                arout, w = arouts[(l, chunk)]
                g_t = pstat.tile([128, w], F32, tag=f"g{l}{chunk}", name=f"g{l}{chunk}")
                nc.sync.dma_start(g_t[:], arout[:])
                gchunk[(l, chunk)] = g_t

            def emit_stats(l, chunk, do_sq, fastl):
                """Per-chunk BN stats on DVE (+ tiny ACT sqrt for general)."""
                g_t = gchunk[(l, chunk)]
                m0 = 0 if chunk == "A" else MAL[l]
                nm = MAL[l] if chunk == "A" else KH - MAL[l]
                ncol = 2 * nm

                def st(tag):
                    return pstat.tile([128, nm], F32, tag=f"{tag}{l}{chunk}", name=f"{tag}{l}{chunk}")

                sg, m1 = st("sg"), st("m1")
                nc.vector.tensor_reduce(
                    sg[:], g_t[:, 0:ncol].rearrange("p (m n) -> p m n", n=2),
                    axis=mybir.AxisListType.X, op=ALU.add)
                nc.vector.tensor_scalar_mul(m1[:], sg[:], 1.0 / B)
                if fastl and not do_sq:
                    negm = st("negm")
                    nc.vector.tensor_scalar_mul(negm[:], sg[:], -1.0 / B)
                    stats[(l, chunk)] = dict(m1=m1, negm=negm, fast=True)
                    return
                gcol = cons[f"g{l}"][:, m0:m0 + nm]
                btcol = cons[f"bt{l}"][:, m0:m0 + nm]
                qg, msq, m1sq, v, sq, r, rp, mt, c = (
                    st(x) for x in ("qg", "msq", "m1sq", "v", "sq", "r", "rp", "mt", "c"))
                nc.vector.tensor_reduce(
                    qg[:], g_t[:, ncol:2 * ncol].rearrange("p (m n) -> p m n", n=2),
                    axis=mybir.AxisListType.X, op=ALU.add)
                nc.vector.tensor_scalar_mul(msq[:], qg[:], 1.0 / B)
                nc.vector.tensor_tensor(m1sq[:], m1[:], m1[:], op=ALU.mult)
                nc.vector.tensor_tensor(v[:], msq[:], m1sq[:], op=ALU.subtract)
                nc.vector.tensor_scalar_add(v[:], v[:], EPS)
                nc.scalar.activation(sq[:], v[:], ACT.Sqrt)
                nc.vector.reciprocal(r[:], sq[:])
                nc.vector.tensor_tensor(rp[:], gcol, r[:], op=ALU.mult)
                nc.vector.tensor_tensor(mt[:], m1[:], rp[:], op=ALU.mult)
                nc.vector.tensor_tensor(c[:], btcol, mt[:], op=ALU.subtract)
                d = dict(m1=m1, rp=rp, c=c, fast=False)
                if l < 3:
                    # sign thresholds for general path
                    gi, u, u2, tthr, s, s2, sneg = (
                        st(x) for x in ("gi", "u", "u2", "tthr", "s", "s2", "sneg"))
                    nc.vector.reciprocal(gi[:], gcol)
                    nc.vector.tensor_tensor(u[:], btcol, gi[:], op=ALU.mult)
                    nc.vector.tensor_tensor(u2[:], u[:], sq[:], op=ALU.mult)
                    nc.vector.tensor_tensor(tthr[:], m1[:], u2[:], op=ALU.subtract)
                    nc.scalar.activation(s[:], gcol, ACT.Sign)
                    nc.vector.tensor_scalar_mul(s2[:], s[:], 2.0)
                    nc.vector.tensor_scalar_mul(sneg[:], s[:], -1.0)
                    d.update(tthr=tthr, s2=s2, sneg=sneg)
                stats[(l, chunk)] = d

            def sign_wave(l, dst3, h_t, krange):
                """a[:, k, :] = sign-of-bn for k in krange; alternate ACT/DVE."""
                for k in krange:
                    ck = "A" if k < MAL[l] else "B"
                    j = k if k < MAL[l] else k - MAL[l]
                    s = stats[(l, ck)]
                    hsl = h_t[:, k * BS:(k + 1) * BS]
                    dst = dst3[:, k, :]
                    if k % 2 == 1:
                        scale = 1.0 if s["fast"] else s["rp"][:, j:j + 1]
                        bias = s["negm"][:, j:j + 1] if s["fast"] else s["c"][:, j:j + 1]
                        nc.scalar.activation(dst, hsl, ACT.Sign, bias=bias, scale=scale)
                    else:
                        thr = s["m1"][:, j:j + 1] if s["fast"] else s["tthr"][:, j:j + 1]
                        bt_ = pscr.tile([128, BS], F16, tag="scr", name=f"sgb_{l}_{k}")
                        nc.vector.tensor_scalar(out=bt_[:], in0=hsl, scalar1=thr,
                                                scalar2=None, op0=ALU.is_ge)
                        s2a = 2.0 if s["fast"] else s["s2"][:, j:j + 1]
                        sna = -1.0 if s["fast"] else s["sneg"][:, j:j + 1]
                        nc.vector.tensor_scalar(out=dst, in0=bt_[:], scalar1=s2a,
                                                scalar2=sna, op0=ALU.mult, op1=ALU.add)

            def drain(l, m, n, ps, h_t, do_sq):
                hs = h_t[:, m * BS + n * 512: m * BS + n * 512 + 512]
                col = _pcol(l, m, n)
                nc.scalar.activation(hs, ps[:], ACT.Identity,
                                     bias=cons[f"b{l}"][:, m:m + 1], scale=1.0,
                                     accum_out=parts[l][:, col:col + 1])
                if do_sq:
                    scr = pscr.tile([128, 512], F32, tag="scr", name=f"sq_{l}_{m}_{n}")
                    nc.scalar.activation(scr[:], hs, ACT.Square,
                                         accum_out=parts[l][:, 32 + col:32 + col + 1])

            def debug_out(src_ap, cast=False):
                if cast:
                    t = pscr.tile([128, BS], F32, tag="scr", name="dbgcast")
                    nc.vector.tensor_copy(t[:C, :], src_ap)
                    src_ap = t[:C, :]
                nc.sync.dma_start(out_d[:], src_ap)

            wdr_pf = {}

            def prefetch_wdr(l, count):
                for m in range(count):
                    w8t = pwdr.tile([128, KH * 128], F8E4, tag="wdr",
                                    name=f"wdr_{l}_{m}")
                    nc.sync.dma_start(w8t[:], ws8[l][:, m * 2048:(m + 1) * 2048])
                    wdr_pf[(l, m)] = w8t

            # ===================== Layer 1 =====================
            h1 = ph.tile([128, KH * BS], F32, tag="ph", name="h1")
            parts[1] = pstat.tile([128, 64], F32, tag="parts1", name="parts1")
            do_sq1 = not fast[0]
            pre_i = 0
            for m in range(KH):
                # weights: one DMA + in-place sign (bf16), + scaled fp8e5 copy
                wst = pw.tile([128, KD * 128], BF16, tag="w", name=f"w1_{m}")
                nc.sync.dma_start(
                    wst[:].rearrange("p (k c) -> p k c", c=128),
                    w1t_d[:, m * 128:(m + 1) * 128].rearrange("(k p) c -> p k c", p=128),
                )
                nc.scalar.activation(wst[:], wst[:], ACT.Sign)
                w8lo = pw8.tile([128, KD * 128], F8E5, tag="w8", name=f"w8lo_{m}")
                nc.vector.tensor_scalar_mul(w8lo[:], wst[:], 2.0 ** -12)
                w8lov = w8lo[:].rearrange("p (k c) -> p k c", c=128)
                pss = [ppsum.tile([128, 512], F32, tag="ps", name=f"ps1_{m}_{n}")
                       for n in range(NB)]
                for k in range(KD):
                    lhsT = wst[:, k * 128:(k + 1) * 128]
                    for n in range(NB):
                        nc.tensor.matmul(
                            pss[n][:], lhsT,
                            xhi[:, k * BS + n * 512: k * BS + n * 512 + 512],
                            start=(k == 0), stop=False)
                for t in range(KD // 2):
                    lhsT = w8lov[:, 2 * t:2 * t + 2, :]
                    for n in range(NB):
                        nc.tensor.matmul(
                            pss[n][:], lhsT,
                            xlo8v[:, 2 * t:2 * t + 2, n * 512:n * 512 + 512],
                            start=False, stop=(t == KD // 2 - 1), perf_mode=DRM)
                for n in range(NB):
                    drain(1, m, n, pss[n], h1, do_sq1)
                # spread L2/L3 weight-sign prepass through L1
                nu = len(pre_units) // KH
                emit_prepass(pre_units[pre_i:pre_i + nu])
                pre_i += nu
                if m == MAL[1] - 1:
                    emit_ar_fire(1, "A", do_sq1)
            emit_ar_fire(1, "B", do_sq1)
            prefetch_wdr(2, 3)
            emit_ar_land(1, "A")
            emit_ar_land(1, "B")
            # stats emitted after the loop: their AR-gated ops must not sit in
            # the engine FIFOs ahead of trailing per-m work
            emit_stats(1, "A", do_sq1, fast[0])
            emit_stats(1, "B", do_sq1, fast[0])

            if stage == 1:
                debug_out(h1[:C, :BS])

            a2 = pa.tile([128, KH, BS], F8E4, tag="pa", name="a2")
            sign_wave(1, a2, h1, range(0, MAL[1]))
            sign_wave(1, a2, h1, range(MAL[1], KH))
            if stage == 2:
                debug_out(a2[:C, 0, :], cast=True)

            # ===================== Layers 2, 3 =====================
            def dense_dr(l, a_in):
                h_t = ph.tile([128, KH * BS], F32, tag="ph", name=f"h{l}")
                parts[l] = pstat.tile([128, 64], F32, tag=f"parts{l}", name=f"parts{l}")
                do_sq = (l == 3) or not fast[l - 1]
                for m in range(KH):
                    if (l, m) in wdr_pf:
                        w8t = wdr_pf.pop((l, m))
                    else:
                        w8t = pwdr.tile([128, KH * 128], F8E4, tag="wdr",
                                        name=f"wdr_{l}_{m}")
                        nc.sync.dma_start(w8t[:], ws8[l][:, m * 2048:(m + 1) * 2048])
                    w8v = w8t[:].rearrange("p (k c) -> p k c", c=128)
                    pss = [ppsum.tile([128, 512], F32, tag="ps", name=f"ps{l}_{m}_{n}")
                           for n in range(NB)]
                    for t in range(KH // 2):
                        lhsT = w8v[:, 2 * t:2 * t + 2, :]
                        for n in range(NB):
                            nc.tensor.matmul(
                                pss[n][:], lhsT,
                                a_in[:, 2 * t:2 * t + 2, n * 512:n * 512 + 512],
                                start=(t == 0), stop=(t == KH // 2 - 1), perf_mode=DRM)
                    for n in range(NB):
                        drain(l, m, n, pss[n], h_t, do_sq)
                    if m == MAL[l] - 1:
                        emit_ar_fire(l, "A", do_sq)
                emit_ar_fire(l, "B", do_sq)
                if l == 2:
                    prefetch_wdr(3, 3)
                emit_ar_land(l, "A")
                emit_ar_land(l, "B")
                emit_stats(l, "A", do_sq, l < 3 and fast[l - 1])
                emit_stats(l, "B", do_sq, l < 3 and fast[l - 1])
                return h_t

            if stage >= 3:
                h2 = dense_dr(2, a2[:])
                a3 = pb.tile([128, KH, BS], F8E4, tag="pb", name="a3")
                sign_wave(2, a3, h2, range(0, MAL[2]))
                sign_wave(2, a3, h2, range(MAL[2], KH))
                if stage == 3:
                    debug_out(a3[:C, 0, :], cast=True)

            if stage >= 4:
                h3 = dense_dr(3, a3[:])
                # y3 = clip(bn3(h3), -1, 1) in fp16: ACT scale+bias, GPSIMD
                # clip; L4 matmuls interleave per k so the PE follows the wave
                y3 = pa.tile([128, KH * BS], F16, tag="pa", name="y3")
                logits = plog.tile([16, BS], F32, tag="logits")
                ps4 = [ppsum.tile([128, 512], F32, tag="ps", name=f"ps4_{n}")
                       for n in range(NB)]
                for k in range(KH):
                    ck = "A" if k < MAL[3] else "B"
                    j = k if k < MAL[3] else k - MAL[3]
                    s = stats[(3, ck)]
                    scr = pscr.tile([128, BS], F32, tag="scr", name=f"y3s_{k}")
                    nc.scalar.activation(scr[:], h3[:, k * BS:(k + 1) * BS],
                                         ACT.Identity, bias=s["c"][:, j:j + 1],
                                         scale=s["rp"][:, j:j + 1])
                    nc.vector.tensor_scalar(out=y3[:, k * BS:(k + 1) * BS],
                                            in0=scr[:], scalar1=-1.0, scalar2=1.0,
                                            op0=ALU.max, op1=ALU.min)
                    if stage >= 5:
                        for n in range(NB):
                            nc.tensor.matmul(
                                ps4[n][:C, :], w4f[:, k * C:(k + 1) * C],
                                y3[:, k * BS + n * 512: k * BS + n * 512 + 512],
                                start=(k == 0), stop=(k == KH - 1))
                if stage == 4:
                    debug_out(y3[:C, :BS], cast=True)

            if stage >= 5:
                # ===== logits + log-softmax, overlapped per n-chunk =====
                e_t = pscr.tile([128, BS], F32, tag="scr", name="exp")
                lse = pscr.tile([128, BS], F32, tag="scr", name="lse")
                for n in range(NB):
                    nsl = slice(n * 512, (n + 1) * 512)
                    nc.scalar.activation(logits[:C, nsl], ps4[n][:C, :],
                                         ACT.Identity, bias=b4s[:C, :], scale=1.0)
                    nc.scalar.activation(e_t[:C, nsl], logits[:C, nsl], ACT.Exp)
                    ps5 = ppsum.tile([128, 512], F32, tag="ps", name=f"ps5_{n}")
                    nc.tensor.matmul(ps5[:1, :], ones10[:C, :], e_t[:C, nsl],
                                     start=True, stop=True)
                    nc.scalar.activation(lse[:1, nsl], ps5[:1, :], ACT.Ln)
                if stage == 5:
                    debug_out(logits[:C, :])

            if stage >= 6:
                outs = plog.tile([16, BS], F32, tag="outs")
                for n in range(NB):
                    nsl = slice(n * 512, (n + 1) * 512)
                    ps6 = ppsum.tile([128, 512], F32, tag="ps", name=f"ps6_{n}")
                    nc.tensor.matmul(ps6[:C, :], onesC[:1, :C], lse[:1, nsl],
                                     start=True, stop=True)
                    nc.vector.tensor_tensor(outs[:C, nsl], logits[:C, nsl],
                                            ps6[:C, :], op=ALU.subtract)
                nc.sync.dma_start(out_d[:], outs[:C, :])

    nc.compile()
    return nc


def _prep_inputs(x, W1, b1, g1, bt1, W2, b2, g2, bt2, W3, b3, g3, bt3, W4, b4):
    """Host-side sharding + layout prep (layout/permutation + lossless-ish split)."""
    def as32(a):
        return np.ascontiguousarray(np.asarray(a, dtype=np.float32))

    x = as32(x)
    bf = ml_dtypes.bfloat16
    shared = {
        "W1T": np.ascontiguousarray(as32(W1).T).astype(bf),
        "W2T": np.ascontiguousarray(as32(W2).T).astype(bf),
        "W3T": np.ascontiguousarray(as32(W3).T).astype(bf),
    }
    cvecs = (b1, g1, bt1, b2, g2, bt2, b3, g3, bt3)
    cpk = np.empty((128, KH * len(cvecs)), np.float32)
    for i, v in enumerate(cvecs):
        cpk[:, i * KH:(i + 1) * KH] = as32(v).reshape(KH, 128).T
    shared["cpk"] = cpk
    w4T = np.ascontiguousarray(as32(W4).T)          # [H, C]
    w4pk = np.empty((128, C * KH), np.float16)
    for k in range(KH):
        w4pk[:, k * C:(k + 1) * C] = w4T[k * 128:(k + 1) * 128, :].astype(np.float16)
    shared["w4pk"] = w4pk
    b4p = np.zeros((16, 1), np.float32)
    b4p[:C, 0] = as32(b4).reshape(-1)
    shared["c_b4"] = b4p

    in_maps = []
    for c in range(NCORES):
        xT = np.ascontiguousarray(x[c * BS:(c + 1) * BS].T)     # [D, BS]
        hi = xT.astype(np.float16)
        lo8 = ((xT - hi.astype(np.float32)) * 4096.0).astype(ml_dtypes.float8_e4m3)
        m = dict(shared)
        m["xT_hi"] = hi
        m["xT_lo8"] = lo8
        in_maps.append(m)
    return in_maps


def _fast_flags(inputs):
    """Mean-only BN boundary valid when beta==0 and gamma>0."""
    def ok(g, bt):
        g, bt = np.asarray(g), np.asarray(bt)
        return bool(not np.any(bt) and np.all(g > 0))

    return (ok(inputs["g1"], inputs["bt1"]), ok(inputs["g2"], inputs["bt2"]))


def kernel(**inputs) -> np.ndarray:
    from concourse.bass_utils import run_bass_kernel_spmd

    fast = _fast_flags(inputs)
    if _CACHE.get("fast") != fast:
        _CACHE["nc"] = _build(fast=fast)
        _CACHE["fast"] = fast
    nc = _CACHE["nc"]
    in_maps = _prep_inputs(**inputs)
    res = run_bass_kernel_spmd(nc, in_maps, list(range(NCORES)))
    out = np.concatenate([res.results[c]["outT"].T for c in range(NCORES)], axis=0)
    return out.astype(np.float32)



# revision 9
# speedup vs baseline: 1.0334x; 1.0334x over previous
"""Trainium2 Bass kernel for nn_BinarizedCifar10MLP — v3.

Data-parallel over batch (8192/8 = 1024 rows/core), feature-major layout.

vs v2 (573us):
  - All weight signing moved to the HOST: W1 ships as fp8e4 +-1 (6.3MB,
    was 12.6MB bf16 + on-device sign), W2/W3 ship as fp8e4 +-1 in DR
    layout (no bf16 read + sign + DRAM round-trip prepass at all).
  - DMA queue discipline: bulk loads (x, W1, Wdr) ride the ACT hwdge
    queue; the sync queue carries only AR traffic + consts + final out.
    W1 m0 is FIRST on the queue (v2 had it behind all 9.4MB of x -> 43us
    PE stall at start); x chunks interleave with W1 m0 sub-tiles and the
    L1 m-loop consumes x chunk-by-chunk, so the PE rides the DMA wave.
  - 3-chunk BN-stat AllReduce for L2/L3 (m 0-9 / 10-13 / 14-15): AR-A
    fires at ~60% of the (short) layer instead of 87%, landing before
    the layer ends; quad-grouped phase-major matmul emission gives the
    PE a 4-m-tile runway on already-signed k-tiles while the tail AR
    lands. Stat sums are n-pair-reduced BEFORE the AR (half payload).
  - log-softmax tail in 4 chunks of 256 cols, exp computed straight
    from PSUM in parallel with the logits drain (DVE reads PSUM).
"""

import sys

sys.path.insert(0, "/opt/trn_rl_repo")

import numpy as np
import ml_dtypes

B, D, H, C = 8192, 3 * 32 * 32, 2048, 10
EPS = 1e-5
NCORES = 8
BS = B // NCORES          # 1024 batch rows per core
KD = D // 128             # 24 k-tiles over input dim
KH = H // 128             # 16 k-tiles over hidden dim
NB = BS // 512            # 2 free-dim chunks of 512
CHK = {1: (14, 16), 2: (10, 14, 16), 3: (10, 14, 16)}  # AR chunk end bounds
XCH = 4                   # x DMA chunks (6 k-tiles each)
KC = KD // XCH            # 6 k-tiles per x chunk

_CACHE = {}


def _bounds(l):
    return (0,) + CHK[l]


def _build(stage=7, fast=(False, False)):
    import concourse.bacc as bacc
    import concourse.mybir as mybir
    import concourse.tile as tile

    F32 = mybir.dt.float32
    F16 = mybir.dt.float16
    F8E4 = mybir.dt.float8e4
    F8E5 = mybir.dt.float8e5
    DRM = mybir.MatmulPerfMode.DoubleRow
    ACT = mybir.ActivationFunctionType
    ALU = mybir.AluOpType
    RG = [list(range(NCORES))]

    nc = bacc.Bacc("TRN2", target_bir_lowering=False, debug=False, num_devices=NCORES)

    # ---- I/O ----
    xhi_d = nc.dram_tensor("xT_hi", [128, KD * BS], F16, kind="ExternalInput").ap()
    xlo_d = nc.dram_tensor("xT_lo8", [128, KD * BS], F8E4, kind="ExternalInput").ap()
    w1_d = nc.dram_tensor("w1pk", [128, KH * KD * 128], F8E4, kind="ExternalInput").ap()
    w2_d = nc.dram_tensor("w2pk", [128, KH * KH * 128], F8E4, kind="ExternalInput").ap()
    w3_d = nc.dram_tensor("w3pk", [128, KH * KH * 128], F8E4, kind="ExternalInput").ap()
    CNAMES = ("b1", "g1", "bt1", "b2", "g2", "bt2", "b3", "g3", "bt3")
    cpk_d = nc.dram_tensor("cpk", [128, KH * len(CNAMES)], F32, kind="ExternalInput").ap()
    w4pk_d = nc.dram_tensor("w4pk", [128, C * KH], F16, kind="ExternalInput").ap()
    b4_d = nc.dram_tensor("c_b4", [16, 1], F32, kind="ExternalInput").ap()
    out_d = nc.dram_tensor("outT", [C, BS], F32, kind="ExternalOutput").ap()

    wl_d = {2: w2_d, 3: w3_d}

    with tile.TileContext(nc) as tc:
        with (
            tc.tile_pool(name="pconst", bufs=1) as pconst,
            tc.tile_pool(name="pstat", bufs=1) as pstat,
            tc.tile_pool(name="plog", bufs=1) as plog,
            tc.tile_pool(name="ptail", bufs=6) as ptail,
            tc.tile_pool(name="pscr", bufs=3) as pscr,
            tc.tile_pool(name="pw", bufs=4) as pw,
            tc.tile_pool(name="pw8", bufs=2) as pw8,
            tc.tile_pool(name="pwdr", bufs=8) as pwdr,
            tc.tile_pool(name="py3", bufs=4) as py3,
            tc.tile_pool(name="ph", bufs=1) as ph,
            tc.tile_pool(name="pa", bufs=1) as pa,
            tc.tile_pool(name="pb", bufs=1) as pb,
            tc.tile_pool(name="ppsum", bufs=8, space="PSUM") as ppsum,
            tc.tile_pool(name="pdram", bufs=16, space="DRAM") as pdram,
        ):
            # ---- warmup AllReduce: absorbs ncfw first-collective staging ----
            wuin = pdram.tile([128, 4], F32, tag="wuin")
            wuout = pdram.tile([128, 4], F32, tag="wuout")
            wusrc = pstat.tile([128, 4], F32, tag="wusrc")
            nc.vector.memset(wusrc[:], 0.0)
            nc.sync.dma_start(wuin[:], wusrc[:])
            nc.gpsimd.collective_compute(
                "AllReduce", ALU.add, replica_groups=RG,
                ins=[wuin.opt()], outs=[wuout.opt()])

            # ---- constants (sync queue; small, land early) ----
            cpk = pconst.tile([128, KH * len(CNAMES)], F32, tag="cpk")
            nc.sync.dma_start(cpk[:], cpk_d)
            cons = {name: cpk[:, i * KH:(i + 1) * KH] for i, name in enumerate(CNAMES)}
            b4s = pconst.tile([16, 1], F32, tag="b4")
            nc.sync.dma_start(b4s[:], b4_d)
            ones10 = pconst.tile([16, 1], F32, tag="ones10")
            nc.vector.memset(ones10[:], 1.0)
            onesC = pconst.tile([1, 16], F32, tag="onesC")
            nc.vector.memset(onesC[:], 1.0)

            # ---- bulk loads: scalar (ACT hwdge) queue ----
            # W1 m0 interleaved chunk-wise with x so the PE starts ~5us in.
            xhi = pa.tile([128, KD * BS], F16, tag="pa", name="xhi")
            xlo8 = pb.tile([128, KD * BS], F8E4, tag="pb", name="xlo8")
            w1_pf = {}
            for m in range(3):
                w1_pf[m] = pw.tile([128, KD * 128], F8E4, tag="w", name=f"w1_{m}")
            for c in range(XCH):
                nc.scalar.dma_start(
                    w1_pf[0][:, c * KC * 128:(c + 1) * KC * 128],
                    w1_d[:, c * KC * 128:(c + 1) * KC * 128])
                sl = slice(c * KC * BS, (c + 1) * KC * BS)
                nc.scalar.dma_start(xhi[:, sl], xhi_d[:, sl])
                nc.scalar.dma_start(xlo8[:, sl], xlo_d[:, sl])
            for m in range(1, 3):
                nc.scalar.dma_start(
                    w1_pf[m][:], w1_d[:, m * KD * 128:(m + 1) * KD * 128])
            xlo8v = xlo8[:].rearrange("p (k c) -> p k c", c=BS)

            # Wdr stream: fp8 +-1 DR-layout weights for L2/L3, 8-deep ring.
            # gens 0..15 = L2 m0..15, 16..31 = L3 m0..15.
            wdr_pf = {}

            def emit_wdr(gen):
                l, m = (2, gen) if gen < 16 else (3, gen - 16)
                w8t = pwdr.tile([128, KH * 128], F8E4, tag="wdr", name=f"wdr_{l}_{m}")
                nc.scalar.dma_start(w8t[:], wl_d[l][:, m * 2048:(m + 1) * 2048])
                wdr_pf[(l, m)] = w8t

            for gen in range(3):
                emit_wdr(gen)

            w4f = pconst.tile([128, C * KH], F16, tag="w4f")
            nc.scalar.dma_start(w4f[:], w4pk_d)

            parts = {}
            gchunk = {}     # (l, ci) -> allreduced pre-reduced stats tile
            stats = {}      # (l, ci) -> dict of stat tiles
            arouts = {}

            def emit_ar_fire(l, ci, do_sq):
                """n-pair-reduce parts chunk -> DRAM -> AllReduce."""
                bd = _bounds(l)
                c0, c1 = bd[ci], bd[ci + 1]
                nm = c1 - c0
                w = 2 * nm if do_sq else nm
                red = pstat.tile([128, w], F32, tag=f"red{l}{ci}", name=f"red{l}{ci}")
                nc.vector.tensor_reduce(
                    red[:, 0:nm],
                    parts[l][:, 2 * c0:2 * c1].rearrange("p (m n) -> p m n", n=2),
                    axis=mybir.AxisListType.X, op=ALU.add)
                if do_sq:
                    nc.vector.tensor_reduce(
                        red[:, nm:w],
                        parts[l][:, 32 + 2 * c0:32 + 2 * c1]
                        .rearrange("p (m n) -> p m n", n=2),
                        axis=mybir.AxisListType.X, op=ALU.add)
                arin = pdram.tile([128, w], F32, tag=f"arin{l}{ci}")
                arout = pdram.tile([128, w], F32, tag=f"arout{l}{ci}")
                nc.sync.dma_start(arin[:], red[:])
                nc.gpsimd.collective_compute(
                    "AllReduce", ALU.add, replica_groups=RG,
                    ins=[arin.opt()], outs=[arout.opt()])
                arouts[(l, ci)] = (arout, w)

            def emit_ar_land(l, ci):
                arout, w = arouts[(l, ci)]
                g_t = pstat.tile([128, w], F32, tag=f"g{l}{ci}", name=f"g{l}{ci}")
                nc.sync.dma_start(g_t[:], arout[:])
                gchunk[(l, ci)] = g_t

            def _st(l, ci, tag, nm):
                return pstat.tile([128, nm], F32, tag=f"{tag}{l}{ci}",
                                  name=f"{tag}{l}{ci}")

            def emit_stats_pre(l, ci, do_sq, fastl):
                """DVE-only stats from the pre-reduced AR result (safe to emit
                mid-loop: no ACT ops to block later drains)."""
                g_t = gchunk[(l, ci)]
                bd = _bounds(l)
                nm = bd[ci + 1] - bd[ci]
                m1 = _st(l, ci, "m1", nm)
                nc.vector.tensor_scalar_mul(m1[:], g_t[:, 0:nm], 1.0 / B)
                if fastl and not do_sq:
                    negm = _st(l, ci, "negm", nm)
                    nc.vector.tensor_scalar_mul(negm[:], g_t[:, 0:nm], -1.0 / B)
                    stats[(l, ci)] = dict(m1=m1, negm=negm, fast=True)
                    return
                msq, m1sq, v = (_st(l, ci, x, nm) for x in ("msq", "m1sq", "v"))
                nc.vector.tensor_scalar_mul(msq[:], g_t[:, nm:2 * nm], 1.0 / B)
                nc.vector.tensor_tensor(m1sq[:], m1[:], m1[:], op=ALU.mult)
                nc.vector.tensor_tensor(v[:], msq[:], m1sq[:], op=ALU.subtract)
                nc.vector.tensor_scalar_add(v[:], v[:], EPS)
                stats[(l, ci)] = dict(m1=m1, v=v, fast=False)

            def emit_stats_post(l, ci, fastl):
                """ACT sqrt + downstream scale/bias (emit after the layer's
                drains so the ACT queue never blocks on a pending AR)."""
                d = stats[(l, ci)]
                if d["fast"]:
                    return
                bd = _bounds(l)
                c0 = bd[ci]
                nm = bd[ci + 1] - c0
                gcol = cons[f"g{l}"][:, c0:c0 + nm]
                btcol = cons[f"bt{l}"][:, c0:c0 + nm]
                m1, v = d["m1"], d["v"]
                r, rp, mt, cc = (_st(l, ci, x, nm) for x in ("r", "rp", "mt", "c"))
                sq = _st(l, ci, "sq", nm)
                nc.scalar.activation(sq[:], v[:], ACT.Sqrt)
                nc.vector.reciprocal(r[:], sq[:])
                nc.vector.tensor_tensor(rp[:], gcol, r[:], op=ALU.mult)
                nc.vector.tensor_tensor(mt[:], m1[:], rp[:], op=ALU.mult)
                nc.vector.tensor_tensor(cc[:], btcol, mt[:], op=ALU.subtract)
                d.update(rp=rp, c=cc)
                if l < 3:
                    gi, u, u2, tthr, s, s2, sneg = (
                        _st(l, ci, x, nm)
                        for x in ("gi", "u", "u2", "tthr", "s", "s2", "sneg"))
                    nc.vector.reciprocal(gi[:], gcol)
                    nc.vector.tensor_tensor(u[:], btcol, gi[:], op=ALU.mult)
                    nc.vector.tensor_tensor(u2[:], u[:], sq[:], op=ALU.mult)
                    nc.vector.tensor_tensor(tthr[:], m1[:], u2[:], op=ALU.subtract)
                    nc.scalar.activation(s[:], gcol, ACT.Sign)
                    nc.vector.tensor_scalar_mul(s2[:], s[:], 2.0)
                    nc.vector.tensor_scalar_mul(sneg[:], s[:], -1.0)
                    d.update(tthr=tthr, s2=s2, sneg=sneg)

            def chunk_of(l, k):
                bd = _bounds(l)
                for ci in range(len(bd) - 1):
                    if k < bd[ci + 1]:
                        return ci, k - bd[ci]

            def sign_wave(l, dst3, h_t, krange):
                """a[:, k, :] = sign-of-bn for k in krange; alternate ACT/DVE."""
                for k in krange:
                    ci, j = chunk_of(l, k)
                    s = stats[(l, ci)]
                    hsl = h_t[:, k * BS:(k + 1) * BS]
                    dst = dst3[:, k, :]
                    if k % 2 == 1:
                        scale = 1.0 if s["fast"] else s["rp"][:, j:j + 1]
                        bias = s["negm"][:, j:j + 1] if s["fast"] else s["c"][:, j:j + 1]
                        nc.scalar.activation(dst, hsl, ACT.Sign, bias=bias, scale=scale)
                    else:
                        thr = s["m1"][:, j:j + 1] if s["fast"] else s["tthr"][:, j:j + 1]
                        bt_ = pscr.tile([128, BS], F16, tag="scr", name=f"sgb_{l}_{k}")
                        nc.vector.tensor_scalar(out=bt_[:], in0=hsl, scalar1=thr,
                                                scalar2=None, op0=ALU.is_ge)
                        s2a = 2.0 if s["fast"] else s["s2"][:, j:j + 1]
                        sna = -1.0 if s["fast"] else s["sneg"][:, j:j + 1]
                        nc.vector.tensor_scalar(out=dst, in0=bt_[:], scalar1=s2a,
                                                scalar2=sna, op0=ALU.mult, op1=ALU.add)

            def drain(l, m, n, ps, h_t, do_sq):
                hs = h_t[:, m * BS + n * 512: m * BS + n * 512 + 512]
                col = 2 * m + n
                nc.scalar.activation(hs, ps[:], ACT.Identity,
                                     bias=cons[f"b{l}"][:, m:m + 1], scale=1.0,
                                     accum_out=parts[l][:, col:col + 1])
                if do_sq:
                    scr = pscr.tile([128, 512], F32, tag="scr", name=f"sq_{l}_{m}_{n}")
                    nc.scalar.activation(scr[:], hs, ACT.Square,
                                         accum_out=parts[l][:, 32 + col:32 + col + 1])

            def debug_out(src_ap, cast=False):
                if cast:
                    t = pscr.tile([128, BS], F32, tag="scr", name="dbgcast")
                    nc.vector.tensor_copy(t[:C, :], src_ap)
                    src_ap = t[:C, :]
                nc.sync.dma_start(out_d[:], src_ap)

            # ===================== Layer 1 =====================
            h1 = ph.tile([128, KH * BS], F32, tag="ph", name="h1")
            parts[1] = pstat.tile([128, 64], F32, tag="parts1", name="parts1")
            do_sq1 = not fast[0]
            bd1 = _bounds(1)
            for m in range(KH):
                wst = w1_pf.pop(m)
                w8lo = pw8.tile([128, KD * 128], F8E5, tag="w8", name=f"w8lo_{m}")
                w8lov = w8lo[:].rearrange("p (k c) -> p k c", c=128)
                pss = [ppsum.tile([128, 512], F32, tag="ps", name=f"ps1_{m}_{n}")
                       for n in range(NB)]
                for c in range(XCH):
                    for k in range(c * KC, (c + 1) * KC):
                        lhsT = wst[:, k * 128:(k + 1) * 128]
                        for n in range(NB):
                            nc.tensor.matmul(
                                pss[n][:], lhsT,
                                xhi[:, k * BS + n * 512: k * BS + n * 512 + 512],
                                start=(k == 0), stop=False)
                    nc.vector.tensor_scalar_mul(
                        w8lo[:, c * KC * 128:(c + 1) * KC * 128],
                        wst[:, c * KC * 128:(c + 1) * KC * 128], 2.0 ** -12)
                    for t in range(c * KC // 2, (c + 1) * KC // 2):
                        lhsT = w8lov[:, 2 * t:2 * t + 2, :]
                        for n in range(NB):
                            nc.tensor.matmul(
                                pss[n][:], lhsT,
                                xlo8v[:, 2 * t:2 * t + 2, n * 512:n * 512 + 512],
                                start=False, stop=(t == KD // 2 - 1), perf_mode=DRM)
                # prefetch W1 3 ahead, Wdr gens 3..7 during m=8..12
                if m + 3 < KH:
                    w1_pf[m + 3] = pw.tile([128, KD * 128], F8E4, tag="w",
                                           name=f"w1_{m + 3}")
                    nc.scalar.dma_start(
                        w1_pf[m + 3][:],
                        w1_d[:, (m + 3) * KD * 128:(m + 4) * KD * 128])
                if 8 <= m <= 12:
                    emit_wdr(m - 5)
                for n in range(NB):
                    drain(1, m, n, pss[n], h1, do_sq1)
                for ci in range(len(bd1) - 1):
                    if m == bd1[ci + 1] - 1:
                        emit_ar_fire(1, ci, do_sq1)
                        emit_ar_land(1, ci)
                        emit_stats_pre(1, ci, do_sq1, fast[0])

            if stage == 1:
                debug_out(h1[:C, :BS])

            a2 = pa.tile([128, KH, BS], F8E4, tag="pa", name="a2")
            for ci in range(len(bd1) - 1):
                emit_stats_post(1, ci, fast[0])
                sign_wave(1, a2, h1, range(bd1[ci], bd1[ci + 1]))
            if stage == 2:
                debug_out(a2[:C, 0, :], cast=True)

            # ===================== Layers 2, 3 =====================
            def dense_dr(l, a_in):
                h_t = ph.tile([128, KH * BS], F32, tag="ph", name=f"h{l}")
                parts[l] = pstat.tile([128, 64], F32, tag=f"parts{l}", name=f"parts{l}")
                do_sq = (l == 3) or not fast[l - 1]
                bd = _bounds(l)
                # t-phases matching the PRODUCING layer's sign chunks
                pb_in = _bounds(l - 1)
                tph = [(pb_in[i] // 2, pb_in[i + 1] // 2) for i in range(len(pb_in) - 1)]
                for q in range(KH // 4):
                    ms = range(4 * q, 4 * q + 4)
                    pss = {m: [ppsum.tile([128, 512], F32, tag="ps",
                                          name=f"ps{l}_{m}_{n}") for n in range(NB)]
                           for m in ms}
                    w8 = {m: wdr_pf.pop((l, m)) for m in ms}
                    for ta, tb in tph:
                        for m in ms:
                            w8v = w8[m][:].rearrange("p (k c) -> p k c", c=128)
                            for t in range(ta, tb):
                                lhsT = w8v[:, 2 * t:2 * t + 2, :]
                                for n in range(NB):
                                    nc.tensor.matmul(
                                        pss[m][n][:], lhsT,
                                        a_in[:, 2 * t:2 * t + 2, n * 512:n * 512 + 512],
                                        start=(t == 0), stop=(t == KH // 2 - 1),
                                        perf_mode=DRM)
                    for m in ms:
                        gen = (l - 2) * 16 + m + 8
                        if gen < 32:
                            emit_wdr(gen)
                        for n in range(NB):
                            drain(l, m, n, pss[m][n], h_t, do_sq)
                        for ci in range(len(bd) - 1):
                            if m == bd[ci + 1] - 1:
                                emit_ar_fire(l, ci, do_sq)
                                emit_ar_land(l, ci)
                                emit_stats_pre(l, ci, do_sq, l < 3 and fast[l - 1])
                return h_t

            if stage >= 3:
                h2 = dense_dr(2, a2[:])
                a3 = pb.tile([128, KH, BS], F8E4, tag="pb", name="a3")
                bd2 = _bounds(2)
                for ci in range(len(bd2) - 1):
                    emit_stats_post(2, ci, fast[1])
                    sign_wave(2, a3, h2, range(bd2[ci], bd2[ci + 1]))
                if stage == 3:
                    debug_out(a3[:C, 0, :], cast=True)

            if stage >= 4:
                h3 = dense_dr(3, a3[:])
                # y3 = clip(bn3(h3), -1, 1) in fp16; L4 matmuls follow per k
                logits = plog.tile([16, BS], F32, tag="logits")
                ps4 = [ppsum.tile([128, 512], F32, tag="ps", name=f"ps4_{n}")
                       for n in range(NB)]
                y3dbg = None
                bd3 = _bounds(3)
                for ci in range(len(bd3) - 1):
                    emit_stats_post(3, ci, False)
                    s = stats[(3, ci)]
                    for k in range(bd3[ci], bd3[ci + 1]):
                        j = k - bd3[ci]
                        scr = pscr.tile([128, BS], F32, tag="scr", name=f"y3s_{k}")
                        nc.scalar.activation(scr[:], h3[:, k * BS:(k + 1) * BS],
                                             ACT.Identity, bias=s["c"][:, j:j + 1],
                                             scale=s["rp"][:, j:j + 1])
                        y3k = py3.tile([128, BS], F16, tag="y3", name=f"y3_{k}")
                        nc.vector.tensor_scalar(out=y3k[:], in0=scr[:],
                                                scalar1=-1.0, scalar2=1.0,
                                                op0=ALU.max, op1=ALU.min)
                        if k == 0:
                            y3dbg = y3k
                        if stage >= 5:
                            for n in range(NB):
                                nc.tensor.matmul(
                                    ps4[n][:C, :], w4f[:, k * C:(k + 1) * C],
                                    y3k[:, n * 512:(n + 1) * 512],
                                    start=(k == 0), stop=(k == KH - 1))
                if stage == 4:
                    debug_out(y3dbg[:C, :], cast=True)

            if stage >= 5:
                # ===== logits + log-softmax, 4 chunks of 256 cols =====
                for qq in range(4):
                    bank = ps4[qq // 2]
                    bsl = slice((qq % 2) * 256, (qq % 2) * 256 + 256)
                    qsl = slice(qq * 256, (qq + 1) * 256)
                    # logits on DVE (PSUM read) in parallel with exp on ACT
                    nc.vector.tensor_scalar(out=logits[:C, qsl], in0=bank[:C, bsl],
                                            scalar1=b4s[:C, :], scalar2=None,
                                            op0=ALU.add)
                    e_q = ptail.tile([16, 256], F32, tag="tl", name=f"e_{qq}")
                    nc.scalar.activation(e_q[:C, :], bank[:C, bsl], ACT.Exp,
                                         bias=b4s[:C, :], scale=1.0)
                    ps5 = ppsum.tile([128, 256], F32, tag="ps", name=f"ps5_{qq}")
                    nc.tensor.matmul(ps5[:1, :], ones10[:C, :], e_q[:C, :],
                                     start=True, stop=True)
                    lse_q = ptail.tile([16, 256], F32, tag="tl", name=f"lse_{qq}")
                    nc.scalar.activation(lse_q[:1, :], ps5[:1, :], ACT.Ln)
                    ps6 = ppsum.tile([128, 256], F32, tag="ps", name=f"ps6_{qq}")
                    nc.tensor.matmul(ps6[:C, :], onesC[:1, :C], lse_q[:1, :],
                                     start=True, stop=True)
                    outs_q = ptail.tile([16, 256], F32, tag="tl", name=f"o_{qq}")
                    nc.vector.tensor_tensor(outs_q[:C, :], logits[:C, qsl],
                                            ps6[:C, :], op=ALU.subtract)
                    if stage >= 6:
                        nc.sync.dma_start(out_d[:, qsl], outs_q[:C, :])
                if stage == 5:
                    debug_out(logits[:C, :])

    nc.compile()
    return nc


def _prep_inputs(x, W1, b1, g1, bt1, W2, b2, g2, bt2, W3, b3, g3, bt3, W4, b4):
    """Host-side sharding + layout prep (sign, fp8 cast, p-major packing)."""
    def as32(a):
        return np.ascontiguousarray(np.asarray(a, dtype=np.float32))

    f8 = ml_dtypes.float8_e4m3

    def sgn(w):
        return np.where(np.asarray(w) >= 0, np.float32(1.0), np.float32(-1.0))

    def pack_w(w, kt):
        # [H_out, K] -> [128, (H_out/128) * K] with per-m-tile p-major blocks
        s = sgn(w).reshape(-1, 128, kt, 128)            # [m, c, k, p]
        s = s.transpose(0, 3, 2, 1).reshape(s.shape[0], 128, kt * 128)
        return np.ascontiguousarray(
            s.transpose(1, 0, 2).reshape(128, -1)).astype(f8)

    x = as32(x)
    shared = {
        "w1pk": pack_w(as32(W1), KD),
        "w2pk": pack_w(as32(W2), KH),
        "w3pk": pack_w(as32(W3), KH),
    }
    cvecs = (b1, g1, bt1, b2, g2, bt2, b3, g3, bt3)
    cpk = np.empty((128, KH * len(cvecs)), np.float32)
    for i, v in enumerate(cvecs):
        cpk[:, i * KH:(i + 1) * KH] = as32(v).reshape(KH, 128).T
    shared["cpk"] = cpk
    w4T = np.ascontiguousarray(as32(W4).T)          # [H, C]
    w4pk = np.empty((128, C * KH), np.float16)
    for k in range(KH):
        w4pk[:, k * C:(k + 1) * C] = w4T[k * 128:(k + 1) * 128, :].astype(np.float16)
    shared["w4pk"] = w4pk
    b4p = np.zeros((16, 1), np.float32)
    b4p[:C, 0] = as32(b4).reshape(-1)
    shared["c_b4"] = b4p

    in_maps = []
    for cr in range(NCORES):
        xT = np.ascontiguousarray(x[cr * BS:(cr + 1) * BS].T)     # [D, BS]
        hi = xT.astype(np.float16)
        lo8 = ((xT - hi.astype(np.float32)) * 4096.0).astype(f8)
        # p-major pack: [D, BS] -> [128, KD*BS]
        hi_pk = np.ascontiguousarray(
            hi.reshape(KD, 128, BS).transpose(1, 0, 2).reshape(128, KD * BS))
        lo_pk = np.ascontiguousarray(
            lo8.reshape(KD, 128, BS).transpose(1, 0, 2).reshape(128, KD * BS))
        m = dict(shared)
        m["xT_hi"] = hi_pk
        m["xT_lo8"] = lo_pk
        in_maps.append(m)
    return in_maps


def _fast_flags(inputs):
    """Mean-only BN boundary valid when beta==0 and gamma>0."""
    def ok(g, bt):
        g, bt = np.asarray(g), np.asarray(bt)
        return bool(not np.any(bt) and np.all(g > 0))

    return (ok(inputs["g1"], inputs["bt1"]), ok(inputs["g2"], inputs["bt2"]))


def kernel(**inputs) -> np.ndarray:
    from concourse.bass_utils import run_bass_kernel_spmd

    fast = _fast_flags(inputs)
    if _CACHE.get("fast") != fast:
        _CACHE["nc"] = _build(fast=fast)
        _CACHE["fast"] = fast
    nc = _CACHE["nc"]
    in_maps = _prep_inputs(**inputs)
    res = run_bass_kernel_spmd(nc, in_maps, list(range(NCORES)))
    out = np.concatenate([res.results[c]["outT"].T for c in range(NCORES)], axis=0)
    return out.astype(np.float32)
